# revision 1
# baseline (speedup 1.0000x reference)
"""Gemma2 attention (B=2, S=2048, HID=2304, H=8, KVH=4, D=256, window=1024,
softcap=50) on 8 TRN2 NeuronCores.

Sharding: DP2 (batch) x TP4 (heads). Core c -> batch c//4, TP rank r=c%4 with
Q heads {2r, 2r+1} and KV head r (GQA-aligned). Wo is row-split over the head
dim; the 4 partial outputs per batch are summed on the host.

Device kernel (identical program on all cores, fp16 matmuls / fp32 PSUM):
  - Projections are interleaved with attention per 512-token chunk; QT/KT are
    feature-major with RoPE fused into the PSUM->SBUF eviction, V token-major.
  - Attention per query block qi (key blocks [qi-8, qi] cover the causal
    sliding window): scores -> softcap tanh (ACT) -> additive triangular masks
    on the boundary blocks (DVE) -> exp(50t-50) with fused row-sum (ACT
    accum_out; no rowmax since P is float32r, which keeps fp32 range) ->
    PE-transpose P -> P.T @ V; 1/rowsum is folded into the AV eviction.
  - The PE stream is software-pipelined: the previous block's Wo matmuls are
    emitted between this block's score matmuls and its softmax-dependent
    transposes, so the in-order PE never waits on ACT/DVE.
"""
import sys

import numpy as np

try:
    import concourse.bass  # noqa: F401
except ImportError:
    sys.path.insert(0, "/opt/trn_rl_repo")

H, KVH, D = 8, 4, 256
S, HID = 2048, 2304
B = 2
SCALING = 256.0 ** -0.5
SOFTCAP = 50.0
THETA = 10000.0
WINDOW = 1024

P = 128
KC = HID // P            # 18 contraction chunks for projections
NQB = S // P             # 16 query blocks
NTC = 4                  # token chunks for projections
TCW = S // NTC           # 512
WBLK = WINDOW // P       # 8: kj in [qi-WBLK, qi]
HG_WIDTHS = [512, 512, 512, 512, 256]   # 2304 split for Wo output groups

_CACHED = {}


def _build_nc():
    import concourse.bass as bass
    import concourse.mybir as mybir
    import concourse.tile as tile
    from concourse import bacc
    from concourse.masks import make_identity

    f32 = mybir.dt.float32
    f16 = mybir.dt.float16
    f32r = mybir.dt.float32r
    AF = mybir.ActivationFunctionType

    nc = bacc.Bacc(None, target_bir_lowering=False)

    hT = nc.dram_tensor("hT", [HID, S], f16, kind="ExternalInput")
    wqT = nc.dram_tensor("wqT", [HID, 2 * D], f16, kind="ExternalInput")
    wkT = nc.dram_tensor("wkT", [HID, D], f16, kind="ExternalInput")
    wvT = nc.dram_tensor("wvT", [HID, D], f16, kind="ExternalInput")
    woT = nc.dram_tensor("woT", [2 * D, HID], f16, kind="ExternalInput")
    cosT = nc.dram_tensor("cosT", [P, S], f16, kind="ExternalInput")
    sinT = nc.dram_tensor("sinT", [P, S], f16, kind="ExternalInput")
    out = nc.dram_tensor("out", [S, HID], f32, kind="ExternalOutput")

    hTr = hT.rearrange("(c p) s -> p c s", p=P)
    wqTr = wqT.rearrange("(c p) m -> p c m", p=P)
    wkTr = wkT.rearrange("(c p) m -> p c m", p=P)
    wvTr = wvT.rearrange("(c p) m -> p c m", p=P)
    woTr = woT.rearrange("(c p) m -> p c m", p=P)

    with tile.TileContext(nc) as tc:
        with (
            tc.tile_pool(name="wpool", bufs=1) as wpool,
            tc.tile_pool(name="hpool", bufs=2) as hpool,
            tc.tile_pool(name="qkv", bufs=1) as qkv,
            tc.tile_pool(name="work", bufs=3) as work,
            tc.tile_pool(name="att3", bufs=3) as att3,
            tc.tile_pool(name="sc", bufs=4) as scpool,
            tc.tile_pool(name="psA", bufs=8, space="PSUM") as psA,
        ):
            # ---------------- persistent SBUF ----------------
            wq_sb = wpool.tile([P, KC, 2 * D], f16)
            wk_sb = wpool.tile([P, KC, D], f16)
            wv_sb = wpool.tile([P, KC, D], f16)
            wo_sb = wpool.tile([P, 4, HID], f16)
            cos_sb = wpool.tile([P, S], f16)
            sin_sb = wpool.tile([P, S], f16)
            ident16 = wpool.tile([P, P], f16)
            ident32 = wpool.tile([P, P], f32)
            identR = wpool.tile([P, P], f32r)
            mask_edge = wpool.tile([P, P], f32)
            mask_diag = wpool.tile([P, P], f32)
            negcap = wpool.tile([P, 1], f32)
            nc.gpsimd.memset(negcap[:], -SOFTCAP)

            qt_sb = qkv.tile([P, 4, S], f16)    # QT feature-major
            kt_sb = qkv.tile([P, 2, S], f16)    # KT feature-major
            v_sb = qkv.tile([P, NQB, D], f16)   # V token-major

            # DMA: few large descriptors; first chunk's operands first.
            ht0 = hpool.tile([P, KC, TCW], f16, tag="ht", name="ht0")
            nc.sync.dma_start(ht0[:, 0:2, :], hTr[:, 0:2, 0:TCW])
            nc.sync.dma_start(wq_sb[:, 0:2, :], wqTr[:, 0:2, :])
            nc.sync.dma_start(ht0[:, 2:6, :], hTr[:, 2:6, 0:TCW])
            nc.sync.dma_start(wq_sb[:, 2:6, :], wqTr[:, 2:6, :])
            nc.sync.dma_start(ht0[:, 6:12, :], hTr[:, 6:12, 0:TCW])
            nc.sync.dma_start(wq_sb[:, 6:12, :], wqTr[:, 6:12, :])
            nc.sync.dma_start(ht0[:, 12:KC, :], hTr[:, 12:KC, 0:TCW])
            nc.sync.dma_start(wq_sb[:, 12:KC, :], wqTr[:, 12:KC, :])
            nc.sync.dma_start(wk_sb[:], wkTr[:, :, :])
            nc.sync.dma_start(wv_sb[:], wvTr[:, :, :])
            nc.sync.dma_start(cos_sb[:], cosT[:, :])
            nc.sync.dma_start(sin_sb[:], sinT[:, :])
            nc.sync.dma_start(wo_sb[:], woTr[:, :, :])

            make_identity(nc, ident16[:])
            make_identity(nc, ident32[:])
            nc.vector.tensor_copy(identR[:], ident32[:])
            # additive masks: 0 where allowed, -3 where masked (t in [-1,1],
            # exp(50*(t-3)-50) underflows to exactly 0 in fp32)
            nc.gpsimd.memset(mask_edge[:], 0.0)
            nc.gpsimd.affine_select(   # window edge: keep dj - di - 1 >= 0
                out=mask_edge[:], in_=mask_edge[:],
                compare_op=mybir.AluOpType.is_ge, fill=-3.0,
                base=-1, pattern=[[1, P]], channel_multiplier=-1)
            nc.gpsimd.memset(mask_diag[:], 0.0)
            nc.gpsimd.affine_select(   # causal diag: keep di - dj >= 0
                out=mask_diag[:], in_=mask_diag[:],
                compare_op=mybir.AluOpType.is_ge, fill=-3.0,
                base=0, pattern=[[-1, P]], channel_multiplier=1)

            def rope_pair(ps_lo, ps_hi, dst, m_lo, m_hi, ts):
                tsl = slice(ts * TCW, (ts + 1) * TCW)
                cs, sn = cos_sb[:, tsl], sin_sb[:, tsl]
                t1 = work.tile([P, TCW], f16, tag="rope_t1")
                t2 = work.tile([P, TCW], f16, tag="rope_t2")
                nc.vector.tensor_mul(t1[:], ps_hi[:], sn)
                nc.vector.tensor_mul(t2[:], ps_lo[:], sn)
                lo = dst[:, m_lo, tsl]
                hi = dst[:, m_hi, tsl]
                nc.vector.tensor_mul(lo, ps_lo[:], cs)
                nc.vector.tensor_sub(lo, lo, t1[:])
                nc.vector.tensor_mul(hi, ps_hi[:], cs)
                nc.vector.tensor_add(hi, hi, t2[:])

            def proj_chunk(ts, ht):
                for pair in range(2):
                    pq = [psA.tile([P, 512], f32, tag="bank",
                                   name=f"pq{ts}_{pair}_{i}") for i in range(2)]
                    for i in range(2):
                        m = 2 * pair + i
                        for k in range(KC):
                            nc.tensor.matmul(
                                pq[i][:], wq_sb[:, k, m * P:(m + 1) * P],
                                ht[:, k, :], start=(k == 0), stop=(k == KC - 1))
                    rope_pair(pq[0], pq[1], qt_sb, 2 * pair, 2 * pair + 1, ts)
                pk = [psA.tile([P, 512], f32, tag="bank", name=f"pk{ts}_{i}")
                      for i in range(2)]
                for i in range(2):
                    for k in range(KC):
                        nc.tensor.matmul(
                            pk[i][:], wk_sb[:, k, i * P:(i + 1) * P],
                            ht[:, k, :], start=(k == 0), stop=(k == KC - 1))
                rope_pair(pk[0], pk[1], kt_sb, 0, 1, ts)
                for mt in range(4):
                    pv = psA.tile([P, 512], f32, tag="bank")
                    for k in range(KC):
                        nc.tensor.matmul(
                            pv[:, :D], ht[:, k, mt * P:(mt + 1) * P],
                            wv_sb[:, k, :], start=(k == 0), stop=(k == KC - 1))
                    nc.scalar.copy(v_sb[:, ts * 4 + mt, :], pv[:, :D])

            def emit_wo(prev):
                """Wo partial for the previous query block (5 psum groups)."""
                if prev is None:
                    return
                atT, q0 = prev
                osb = work.tile([P, HID], f32, tag="osb", name=f"osb{q0}")
                hg0 = 0
                for gi, hgw in enumerate(HG_WIDTHS):
                    po = psA.tile([P, 512], f32, tag="bank",
                                  name=f"po{q0}_{gi}")
                    for m in range(4):
                        nc.tensor.matmul(
                            po[:, :hgw], atT[:, m, :],
                            wo_sb[:, m, hg0:hg0 + hgw],
                            start=(m == 0), stop=(m == 3))
                    if gi % 2 == 0:
                        nc.vector.tensor_copy(osb[:, hg0:hg0 + hgw],
                                              po[:, :hgw])
                    else:
                        nc.scalar.copy(osb[:, hg0:hg0 + hgw], po[:, :hgw])
                    hg0 += hgw
                nc.sync.dma_start(out[q0:q0 + P, :], osb[:])

            def emit_scores(qi):
                kj0 = max(0, qi - WBLK)
                nkb = qi - kj0 + 1
                nk = nkb * P
                qsl = slice(qi * P, (qi + 1) * P)
                tbufs = []
                for h in range(2):
                    tbuf = scpool.tile([P, 9 * P], f32, tag="tbuf",
                                       name=f"tbuf{qi}_{h}")
                    for g0 in range(0, nk, 512):
                        gw = min(512, nk - g0)
                        ps = psA.tile([P, 512], f32, tag="bank",
                                      name=f"ps{qi}_{h}_{g0}")
                        ksl = slice(kj0 * P + g0, kj0 * P + g0 + gw)
                        for i in range(2):
                            nc.tensor.matmul(
                                ps[:, :gw], qt_sb[:, 2 * h + i, qsl],
                                kt_sb[:, i, ksl], start=(i == 0), stop=(i == 1))
                        nc.scalar.activation(
                            tbuf[:, g0:g0 + gw], ps[:, :gw], AF.Tanh,
                            scale=SCALING / SOFTCAP)
                    tbufs.append(tbuf)
                return (qi, tbufs)

            def finish_block(pend):
                qi, tbufs = pend
                kj0 = max(0, qi - WBLK)
                nkb = qi - kj0 + 1
                nk = nkb * P
                at_qi = work.tile([P, 2 * D], f16, tag="at_qi")
                pav = psA.tile([P, 512], f32, tag="bank", name=f"pav{qi}")
                for h in range(2):
                    tbuf = tbufs[h]
                    if kj0 == qi - WBLK:
                        nc.vector.tensor_add(tbuf[:, :P], tbuf[:, :P],
                                             mask_edge[:])
                    dsl = slice((nkb - 1) * P, nkb * P)
                    nc.vector.tensor_add(tbuf[:, dsl], tbuf[:, dsl],
                                         mask_diag[:])
                    negm = scpool.tile([P, 1], f32, tag="negm")
                    nc.vector.tensor_reduce(
                        out=negm[:], in_=tbuf[:, :nk], op=mybir.AluOpType.max,
                        axis=mybir.AxisListType.X, negate=True)
                    negm50 = scpool.tile([P, 1], f32, tag="negm50")
                    nc.vector.tensor_scalar_mul(negm50[:], negm[:], SOFTCAP)
                    pbuf = scpool.tile([P, 9 * P], f16, tag="pbuf",
                                       name=f"pbuf{qi}_{h}")
                    sums = scpool.tile([P, 1], f32, tag="sums")
                    nc.scalar.activation(
                        pbuf[:, :nk], tbuf[:, :nk], AF.Exp,
                        bias=negm50[:], scale=SOFTCAP, accum_out=sums[:])
                    recip = scpool.tile([P, 1], f32, tag="recip")
                    nc.vector.reciprocal(recip[:], sums[:])
                    pt = work.tile([P, 9, P], f16, tag="pt")
                    nb = 0
                    for b0 in range(0, nkb, 4):
                        bw = min(4, nkb - b0)
                        ptp = psA.tile([P, 512], f16, tag="bank",
                                       name=f"ptp{qi}_{h}_{b0}")
                        for j in range(bw):
                            nc.tensor.transpose(
                                ptp[:, j * P:(j + 1) * P],
                                pbuf[:, (b0 + j) * P:(b0 + j + 1) * P],
                                ident16[:])
                        if nb % 2 == 0:
                            nc.vector.tensor_copy(
                                pt[:, b0:b0 + bw, :], ptp[:, :bw * P])
                        else:
                            nc.scalar.copy(
                                pt[:, b0:b0 + bw, :], ptp[:, :bw * P])
                        nb += 1
                    for j in range(nkb):
                        nc.tensor.matmul(
                            pav[:, h * D:h * D + D], pt[:, j, :],
                            v_sb[:, kj0 + j, :],
                            start=(j == 0), stop=(j == nkb - 1))
                    nc.vector.tensor_scalar_mul(
                        at_qi[:, h * D:h * D + D], pav[:, h * D:h * D + D],
                        recip[:])

                # attnT for this token block
                att = psA.tile([P, 512], f16, tag="bank", name=f"att{qi}")
                for m in range(4):
                    nc.tensor.transpose(
                        att[:, m * P:(m + 1) * P], at_qi[:, m * P:(m + 1) * P],
                        ident16[:])
                atT = att3.tile([P, 4, P], f16, tag="atT", name=f"atT{qi}")
                nc.scalar.copy(atT[:], att[:])
                return (atT, qi * P)

            # ---------------- merged pipeline (2-deep) ----------------
            prev = None      # (atT, q0) finished, awaiting Wo
            pend = None      # (qi, tbufs) scored, awaiting softmax/AV
            for ts in range(NTC):
                if ts == 0:
                    ht = ht0
                else:
                    ht = hpool.tile([P, KC, TCW], f16, tag="ht", name=f"ht{ts}")
                    nc.sync.dma_start(ht[:], hTr[:, :, ts * TCW:(ts + 1) * TCW])
                proj_chunk(ts, ht)
                for qi in range(4 * ts, 4 * ts + 4):
                    sc = emit_scores(qi)
                    emit_wo(prev)
                    prev = None
                    if pend is not None:
                        prev = finish_block(pend)
                    pend = sc
            emit_wo(prev)
            prev = finish_block(pend)
            emit_wo(prev)

    nc.compile()
    return nc


def _get_nc():
    if "nc" not in _CACHED:
        _CACHED["nc"] = _build_nc()
    return _CACHED["nc"]


def kernel(hidden_states, Wq, Wk, Wv, Wo, position_ids):
    from concourse.bass_utils import run_bass_kernel_spmd

    hidden_states = np.asarray(hidden_states)
    Wq, Wk, Wv, Wo = (np.asarray(a) for a in (Wq, Wk, Wv, Wo))
    position_ids = np.asarray(position_ids)

    inv_freq = 1.0 / (THETA ** (np.arange(0, D, 2, dtype=np.float64) / D))
    freqs = position_ids.astype(np.float64)[None, :] * inv_freq[:, None]
    cos_t = np.cos(freqs).astype(np.float16)
    sin_t = np.sin(freqs).astype(np.float16)

    in_maps = []
    for c in range(8):
        b, r = divmod(c, 4)
        in_maps.append({
            "hT": np.ascontiguousarray(hidden_states[b].T).astype(np.float16),
            "wqT": np.ascontiguousarray(Wq[512 * r:512 * (r + 1)].T).astype(np.float16),
            "wkT": np.ascontiguousarray(Wk[256 * r:256 * (r + 1)].T).astype(np.float16),
            "wvT": np.ascontiguousarray(Wv[256 * r:256 * (r + 1)].T).astype(np.float16),
            "woT": np.ascontiguousarray(Wo[:, 512 * r:512 * (r + 1)].T).astype(np.float16),
            "cosT": cos_t,
            "sinT": sin_t,
        })

    _CACHED["last_in_maps"] = in_maps
    globals()["_last_in_maps"] = in_maps
    res = run_bass_kernel_spmd(_get_nc(), in_maps, core_ids=list(range(8)))
    parts = [r["out"] for r in res.results]
    full = np.stack([
        parts[0] + parts[1] + parts[2] + parts[3],
        parts[4] + parts[5] + parts[6] + parts[7],
    ]).astype(np.float32)
    return full



# revision 7
# speedup vs baseline: 1.1328x; 1.1328x over previous
"""Gemma2 attention (B=2, S=2048, HID=2304, H=8, KVH=4, D=256, window=1024,
softcap=50) on 8 TRN2 NeuronCores.

Sharding: DP2 (batch) x TP4 (heads). Core c -> batch c//4, TP rank r=c%4 with
Q heads {2r, 2r+1} and KV head r (GQA-aligned). Wo is row-split over the head
dim; the 4 partial outputs per batch are summed on the host.

Device kernel v2 (identical program on all cores):
  - Scores are computed TRANSPOSED ([keys, queries]) for query-block PAIRS
    (256 queries wide), so the softmax weights come out of the exp already in
    the layout AV needs as the stationary operand -- no PE transposes of P.
  - No rowmax: weights = exp(50*tanh(s/50) - 50) <= 1 stored in bf16, whose
    dynamic range (down to ~1e-38) covers any realizable row maximum.
  - Row sums come from a ones-column appended to V (AV matmuls are N=257);
    1/rowsum is folded into the AV eviction (per-partition scalar).
  - Masks are additive -3 tiles applied pre-exp (exp then underflows to 0).
  - Host-side input relayout gives every DMA 128 large contiguous
    per-partition descriptors; output partials are fp16.
  - Dummy matmuls warm the PE (HAM un-throttle) during the initial DMA wait.
"""
import sys

import numpy as np

try:
    import concourse.bass  # noqa: F401
except ImportError:
    sys.path.insert(0, "/opt/trn_rl_repo")

H, KVH, D = 8, 4, 256
S, HID = 2048, 2304
B = 2
SCALING = 256.0 ** -0.5
SOFTCAP = 50.0
THETA = 10000.0
WINDOW = 1024

P = 128
KC = HID // P            # 18 contraction chunks for projections
NTC = 4                  # token chunks for projections
TCW = S // NTC           # 512
NPAIR = 8                # query-block pairs (256 queries each)
HG_WIDTHS = [512, 512, 512, 512, 256]   # 2304 split for Wo output groups

_CACHED = {}


def _build_nc():
    import concourse.bass as bass
    import concourse.mybir as mybir
    import concourse.tile as tile
    from concourse import bacc
    from concourse.masks import make_identity

    f32 = mybir.dt.float32
    f16 = mybir.dt.float16
    bf16 = mybir.dt.bfloat16
    AF = mybir.ActivationFunctionType

    nc = bacc.Bacc(None, target_bir_lowering=False)

    h4 = nc.dram_tensor("h4", [P, NTC, KC, TCW], f16, kind="ExternalInput")
    wq4 = nc.dram_tensor("wq4", [P, KC, 2 * D], f16, kind="ExternalInput")
    wk4 = nc.dram_tensor("wk4", [P, KC, D], f16, kind="ExternalInput")
    wv4 = nc.dram_tensor("wv4", [P, KC, D], f16, kind="ExternalInput")
    wo4 = nc.dram_tensor("wo4", [P, 4, HID], bf16, kind="ExternalInput")
    cosT = nc.dram_tensor("cosT", [P, S], f16, kind="ExternalInput")
    sinT = nc.dram_tensor("sinT", [P, S], f16, kind="ExternalInput")
    out = nc.dram_tensor("out", [S, HID], f16, kind="ExternalOutput")

    with tile.TileContext(nc) as tc:
        with (
            tc.tile_pool(name="wpool", bufs=1) as wpool,
            tc.tile_pool(name="hpool", bufs=2) as hpool,
            tc.tile_pool(name="qkv", bufs=1) as qkv,
            tc.tile_pool(name="work", bufs=2) as work,
            tc.tile_pool(name="att3", bufs=4) as att3,
            tc.tile_pool(name="sc", bufs=1) as scpool,
            tc.tile_pool(name="ptp", bufs=2) as ptpool,
            tc.tile_pool(name="small", bufs=4) as small,
            tc.tile_pool(name="psA", bufs=8, space="PSUM") as psA,
        ):
            # ---------------- persistent SBUF ----------------
            wq_sb = wpool.tile([P, KC, 2 * D], f16)
            wk_sb = wpool.tile([P, KC, D], f16)
            wv_sb = wpool.tile([P, KC, D], f16)
            wo_sb = wpool.tile([P, 4, HID], bf16)
            cos_sb = wpool.tile([P, S], f16)
            sin_sb = wpool.tile([P, S], f16)
            ident_bf = wpool.tile([P, P], bf16)
            wtile = wpool.tile([P, P], f16)
            negb = wpool.tile([P, 1], f32)
            mA = wpool.tile([P, 2, 256], f32)
            mB = wpool.tile([P, 2, 256], f32)
            mC = wpool.tile([P, 2, 256], f32)
            mD = wpool.tile([P, 2, 256], f32)

            qt_sb = qkv.tile([P, 4, S], f16)      # QT feature-major
            kt_sb = qkv.tile([P, 2, S], f16)      # KT feature-major
            vE_sb = qkv.tile([P, 16, D + 1], bf16)  # V token-major + ones col

            # DMA: large per-partition-contiguous descriptors; the operands of
            # the very first matmuls (k-chunks 0:2 of ts=0) land first.
            ht0 = hpool.tile([P, KC, TCW], f16, tag="ht", name="ht0")
            nc.sync.dma_start(ht0[:, 0:2, :], h4[:, 0, 0:2, :])
            nc.sync.dma_start(wq_sb[:, 0:2, :], wq4[:, 0:2, :])
            nc.sync.dma_start(ht0[:, 2:KC, :], h4[:, 0, 2:KC, :])
            nc.sync.dma_start(wq_sb[:, 2:KC, :], wq4[:, 2:KC, :])
            nc.sync.dma_start(wk_sb[:], wk4[:, :, :])
            nc.sync.dma_start(wv_sb[:], wv4[:, :, :])
            nc.sync.dma_start(cos_sb[:], cosT[:, :])
            nc.sync.dma_start(sin_sb[:], sinT[:, :])
            nc.sync.dma_start(wo_sb[:], wo4[:, :, :])

            # small one-time SBUF setup (gpsimd) + PE warm-up during DMA wait
            make_identity(nc, ident_bf[:])
            nc.gpsimd.memset(wtile[:], 0.0)
            nc.gpsimd.memset(negb[:], -SOFTCAP)
            nc.gpsimd.memset(vE_sb[:, :, D], 1.0)
            # transposed-orientation additive masks (0 keep / -3 mask):
            # value(p=dk, head j, f=dq-in-pair) independent of head (coef 0).
            for m_t, base, csign in (
                (mA, -1, 1),     # oldest block: edge for half A, all-mask B
                (mB, 127, 1),    # second block: keep A, edge for half B
                (mC, 0, -1),     # diag block of half A; keep for half B
                (mD, -128, -1),  # diag block of half B; all-mask for half A
            ):
                nc.gpsimd.memset(m_t[:], 0.0)
                nc.gpsimd.affine_select(
                    out=m_t[:], in_=m_t[:],
                    compare_op=mybir.AluOpType.is_ge, fill=-3.0,
                    base=base, pattern=[[0, 2], [-csign, 256]],
                    channel_multiplier=csign)
            warm_ps = psA.tile([P, 512], f32, tag="bank", name="warm")
            for i in range(16):
                nc.tensor.matmul(warm_ps[:, 0:P], wtile[:], wtile[:],
                                 start=True, stop=True)

            def rope_pair(ps_lo, ps_hi, dst, m_lo, m_hi, ts):
                tsl = slice(ts * TCW, (ts + 1) * TCW)
                cs, sn = cos_sb[:, tsl], sin_sb[:, tsl]
                t1 = work.tile([P, TCW], f16, tag="rope_t1")
                t2 = work.tile([P, TCW], f16, tag="rope_t2")
                nc.vector.tensor_mul(t1[:], ps_hi[:], sn)
                nc.vector.tensor_mul(t2[:], ps_lo[:], sn)
                lo = dst[:, m_lo, tsl]
                hi = dst[:, m_hi, tsl]
                nc.vector.tensor_mul(lo, ps_lo[:], cs)
                nc.vector.tensor_sub(lo, lo, t1[:])
                nc.vector.tensor_mul(hi, ps_hi[:], cs)
                nc.vector.tensor_add(hi, hi, t2[:])

            def proj_chunk(ts, ht):
                for pair in range(2):
                    pq = [psA.tile([P, 512], f32, tag="bank",
                                   name=f"pq{ts}_{pair}_{i}") for i in range(2)]
                    for i in range(2):
                        m = 2 * pair + i
                        for k in range(KC):
                            nc.tensor.matmul(
                                pq[i][:], wq_sb[:, k, m * P:(m + 1) * P],
                                ht[:, k, :], start=(k == 0), stop=(k == KC - 1))
                    rope_pair(pq[0], pq[1], qt_sb, 2 * pair, 2 * pair + 1, ts)
                pk = [psA.tile([P, 512], f32, tag="bank", name=f"pk{ts}_{i}")
                      for i in range(2)]
                for i in range(2):
                    for k in range(KC):
                        nc.tensor.matmul(
                            pk[i][:], wk_sb[:, k, i * P:(i + 1) * P],
                            ht[:, k, :], start=(k == 0), stop=(k == KC - 1))
                rope_pair(pk[0], pk[1], kt_sb, 0, 1, ts)
                for mt in range(4):
                    pv = psA.tile([P, 512], f32, tag="bank")
                    for k in range(KC):
                        nc.tensor.matmul(
                            pv[:, :D], ht[:, k, mt * P:(mt + 1) * P],
                            wv_sb[:, k, :], start=(k == 0), stop=(k == KC - 1))
                    nc.scalar.copy(vE_sb[:, ts * 4 + mt, 0:D], pv[:, :D])

            def emit_wo(prev, last=False):
                """Wo partials for finished query blocks [(atT, q0), ...]."""
                if prev is None:
                    return
                for atT, q0 in prev:
                    osb = work.tile([P, HID], f16, tag="osb", name=f"osb{q0}")
                    hg0 = 0
                    for gi, hgw in enumerate(HG_WIDTHS):
                        po = psA.tile([P, 512], f32, tag="bank",
                                      name=f"po{q0}_{gi}")
                        for m in range(4):
                            nc.tensor.matmul(
                                po[:, :hgw], atT[:, m, :],
                                wo_sb[:, m, hg0:hg0 + hgw],
                                start=(m == 0), stop=(m == 3))
                        if gi % 2 == 0:
                            nc.vector.tensor_copy(osb[:, hg0:hg0 + hgw],
                                                  po[:, :hgw])
                        else:
                            nc.scalar.copy(osb[:, hg0:hg0 + hgw], po[:, :hgw])
                        if last:
                            nc.sync.dma_start(out[q0:q0 + P, hg0:hg0 + hgw],
                                              osb[:, hg0:hg0 + hgw])
                        hg0 += hgw
                    if not last:
                        nc.sync.dma_start(out[q0:q0 + P, :], osb[:])

            def emit_scores_pair(p):
                """Transposed softcapped scores for query blocks 2p, 2p+1."""
                j0 = max(0, 2 * p - 8)
                nkb = 2 * p + 2 - j0
                qsl = slice(2 * p * P, (2 * p + 2) * P)
                tbufT = scpool.tile([P, 10, 2, 256], f32, tag="tbufT")
                ptb = ptpool.tile([P, 10, 2, 256], bf16, tag="ptb",
                                  name=f"ptb{p}")
                for jl in range(nkb):
                    kb = j0 + jl
                    ps = psA.tile([P, 2, 256], f32, tag="bank",
                                  name=f"ps{p}_{jl}")
                    psv = ps
                    for h in range(2):
                        for i in range(2):
                            nc.tensor.matmul(
                                psv[:, h, :],
                                kt_sb[:, i, kb * P:(kb + 1) * P],
                                qt_sb[:, 2 * h + i, qsl],
                                start=(i == 0), stop=(i == 1))
                    nc.scalar.activation(
                        tbufT[:, jl, :, :], psv[:, :, :], AF.Tanh,
                        scale=SCALING / SOFTCAP)
                if j0 == 2 * p - 8:
                    nc.vector.tensor_add(tbufT[:, 0], tbufT[:, 0], mA[:])
                    nc.vector.tensor_add(tbufT[:, 1], tbufT[:, 1], mB[:])
                nc.vector.tensor_add(tbufT[:, nkb - 2], tbufT[:, nkb - 2],
                                     mC[:])
                nc.vector.tensor_add(tbufT[:, nkb - 1], tbufT[:, nkb - 1],
                                     mD[:])
                nc.scalar.activation(
                    ptb[:, 0:nkb], tbufT[:, 0:nkb], AF.Exp,
                    scale=SOFTCAP, bias=negb[:])
                return (p, j0, nkb, ptb)

            def finish_pair(pend):
                """AV + normalization + attnT for both halves of a pair."""
                p, j0, nkb, ptb = pend
                at = work.tile([P, 2, 2 * D], bf16, tag="at", name=f"at{p}")
                res = []
                for s in range(2):
                    qi = 2 * p + s
                    kb_lo = max(0, qi - 8)
                    nb = qi - kb_lo + 1
                    for h in range(2):
                        pav = psA.tile([P, 512], f32, tag="bank",
                                       name=f"pav{p}_{s}_{h}")
                        for n in range(nb):
                            kb = kb_lo + n
                            nc.tensor.matmul(
                                pav[:, 0:D + 1],
                                ptb[:, kb - j0, h, s * P:(s + 1) * P],
                                vE_sb[:, kb, :],
                                start=(n == 0), stop=(n == nb - 1))
                        recip = small.tile([P, 1], f32, tag="recip")
                        nc.vector.reciprocal(recip[:], pav[:, D:D + 1])
                        nc.vector.tensor_scalar_mul(
                            at[:, s, h * D:(h + 1) * D], pav[:, 0:D],
                            recip[:])
                    attps = psA.tile([P, 512], bf16, tag="bank",
                                     name=f"att{p}_{s}")
                    for m in range(4):
                        nc.tensor.transpose(
                            attps[:, m * P:(m + 1) * P],
                            at[:, s, m * P:(m + 1) * P], ident_bf[:])
                    atT = att3.tile([P, 4, P], bf16, tag="atT",
                                    name=f"atT{p}_{s}")
                    nc.scalar.copy(atT[:], attps[:])
                    res.append((atT, qi * P))
                return res

            # ---------------- merged pipeline (2-deep) ----------------
            prev = None      # finished halves awaiting Wo
            pend = None      # scored pair awaiting softmax/AV
            ht_cur = ht0
            ht_next = None
            for p in range(NPAIR):
                if p % 2 == 0:
                    ts = p // 2
                    if ts + 1 < NTC:
                        ht_next = hpool.tile([P, KC, TCW], f16, tag="ht",
                                             name=f"ht{ts + 1}")
                        nc.sync.dma_start(ht_next[:],
                                          h4[:, ts + 1, :, :])
                    proj_chunk(ts, ht_cur)
                    ht_cur = ht_next
                sc = emit_scores_pair(p)
                emit_wo(prev)
                prev = None
                if pend is not None:
                    prev = finish_pair(pend)
                pend = sc
            emit_wo(prev)
            prev = finish_pair(pend)
            emit_wo(prev, last=True)

    nc.compile()
    return nc


def _get_nc():
    if "nc" not in _CACHED:
        _CACHED["nc"] = _build_nc()
    return _CACHED["nc"]


def kernel(hidden_states, Wq, Wk, Wv, Wo, position_ids):
    import ml_dtypes
    from concourse.bass_utils import run_bass_kernel_spmd

    hidden_states = np.asarray(hidden_states)
    Wq, Wk, Wv, Wo = (np.asarray(a) for a in (Wq, Wk, Wv, Wo))
    position_ids = np.asarray(position_ids)

    inv_freq = 1.0 / (THETA ** (np.arange(0, D, 2, dtype=np.float64) / D))
    freqs = position_ids.astype(np.float64)[None, :] * inv_freq[:, None]
    cos_t = np.cos(freqs).astype(np.float16)
    sin_t = np.sin(freqs).astype(np.float16)

    in_maps = []
    for c in range(8):
        b, r = divmod(c, 4)
        hT = hidden_states[b].T.astype(np.float16)          # [HID, S]
        h4 = np.ascontiguousarray(
            hT.reshape(KC, P, NTC, TCW).transpose(1, 2, 0, 3))
        wq4 = np.ascontiguousarray(
            Wq[512 * r:512 * (r + 1)].T.astype(np.float16)
            .reshape(KC, P, 2 * D).transpose(1, 0, 2))
        wk4 = np.ascontiguousarray(
            Wk[256 * r:256 * (r + 1)].T.astype(np.float16)
            .reshape(KC, P, D).transpose(1, 0, 2))
        wv4 = np.ascontiguousarray(
            Wv[256 * r:256 * (r + 1)].T.astype(np.float16)
            .reshape(KC, P, D).transpose(1, 0, 2))
        wo4 = np.ascontiguousarray(
            Wo[:, 512 * r:512 * (r + 1)].T.astype(ml_dtypes.bfloat16)
            .reshape(4, P, HID).transpose(1, 0, 2))
        in_maps.append({
            "h4": h4, "wq4": wq4, "wk4": wk4, "wv4": wv4, "wo4": wo4,
            "cosT": cos_t, "sinT": sin_t,
        })

    _CACHED["last_in_maps"] = in_maps
    globals()["_last_in_maps"] = in_maps
    res = run_bass_kernel_spmd(_get_nc(), in_maps, core_ids=list(range(8)))
    parts = [r["out"].astype(np.float32) for r in res.results]
    full = np.stack([
        parts[0] + parts[1] + parts[2] + parts[3],
        parts[4] + parts[5] + parts[6] + parts[7],
    ])
    return full


# revision 11
# speedup vs baseline: 1.1372x; 1.0040x over previous
"""Gemma2 attention (B=2, S=2048, HID=2304, H=8, KVH=4, D=256, window=1024,
softcap=50) on 8 TRN2 NeuronCores.

Sharding: DP2 (batch) x TP4 (heads). Core c -> batch c//4, TP rank r=c%4 with
Q heads {2r, 2r+1} and KV head r (GQA-aligned). Wo is row-split over the head
dim; the 4 partial outputs per batch are summed on the host.

Device kernel v2 (identical program on all cores):
  - Scores are computed TRANSPOSED ([keys, queries]) for query-block PAIRS
    (256 queries wide), so the softmax weights come out of the exp already in
    the layout AV needs as the stationary operand -- no PE transposes of P.
  - No rowmax: weights = exp(50*tanh(s/50) - 50) <= 1 stored in bf16, whose
    dynamic range (down to ~1e-38) covers any realizable row maximum.
  - Row sums come from a ones-column appended to V (AV matmuls are N=257);
    1/rowsum is folded into the AV eviction (per-partition scalar).
  - Masks are additive -3 tiles applied pre-exp (exp then underflows to 0).
  - Host-side input relayout gives every DMA 128 large contiguous
    per-partition descriptors; output partials are fp16.
  - Dummy matmuls warm the PE (HAM un-throttle) during the initial DMA wait.
"""
import sys

import numpy as np

try:
    import concourse.bass  # noqa: F401
except ImportError:
    sys.path.insert(0, "/opt/trn_rl_repo")

H, KVH, D = 8, 4, 256
S, HID = 2048, 2304
B = 2
SCALING = 256.0 ** -0.5
SOFTCAP = 50.0
THETA = 10000.0
WINDOW = 1024

P = 128
KC = HID // P            # 18 contraction chunks for projections
NTC = 4                  # token chunks for projections
TCW = S // NTC           # 512
NPAIR = 8                # query-block pairs (256 queries each)
HG_WIDTHS = [512, 512, 512, 512, 256]   # 2304 split for Wo output groups

_CACHED = {}


def _build_nc():
    import concourse.bass as bass
    import concourse.mybir as mybir
    import concourse.tile as tile
    from concourse import bacc
    from concourse.masks import make_identity

    f32 = mybir.dt.float32
    f16 = mybir.dt.float16
    bf16 = mybir.dt.bfloat16
    AF = mybir.ActivationFunctionType

    nc = bacc.Bacc(None, target_bir_lowering=False)

    h4 = nc.dram_tensor("h4", [P, NTC, KC, TCW], f16, kind="ExternalInput")
    wq4 = nc.dram_tensor("wq4", [P, KC, 2 * D], f16, kind="ExternalInput")
    wk4 = nc.dram_tensor("wk4", [P, KC, D], f16, kind="ExternalInput")
    wv4 = nc.dram_tensor("wv4", [P, KC, D], f16, kind="ExternalInput")
    wo4 = nc.dram_tensor("wo4", [P, 4, HID], bf16, kind="ExternalInput")
    cosT = nc.dram_tensor("cosT", [P, S], f16, kind="ExternalInput")
    sinT = nc.dram_tensor("sinT", [P, S], f16, kind="ExternalInput")
    out = nc.dram_tensor("out", [S, HID], f16, kind="ExternalOutput")

    with tile.TileContext(nc) as tc:
        with (
            tc.tile_pool(name="wpool", bufs=1) as wpool,
            tc.tile_pool(name="hpool", bufs=2) as hpool,
            tc.tile_pool(name="qkv", bufs=1) as qkv,
            tc.tile_pool(name="work", bufs=2) as work,
            tc.tile_pool(name="att3", bufs=4) as att3,
            tc.tile_pool(name="sc", bufs=1) as scpool,
            tc.tile_pool(name="ptp", bufs=2) as ptpool,
            tc.tile_pool(name="small", bufs=4) as small,
            tc.tile_pool(name="psA", bufs=8, space="PSUM") as psA,
        ):
            # ---------------- persistent SBUF ----------------
            wq_sb = wpool.tile([P, KC, 2 * D], f16)
            wk_sb = wpool.tile([P, KC, D], f16)
            wv_sb = wpool.tile([P, KC, D], f16)
            wo_sb = wpool.tile([P, 4, HID], bf16)
            cos_sb = wpool.tile([P, S], f16)
            sin_sb = wpool.tile([P, S], f16)
            ident_bf = wpool.tile([P, P], bf16)
            wtile = wpool.tile([P, P], f16)
            negb = wpool.tile([P, 1], f32)
            mA = wpool.tile([P, 2, 256], f32)
            mB = wpool.tile([P, 2, 256], f32)
            mC = wpool.tile([P, 2, 256], f32)
            mD = wpool.tile([P, 2, 256], f32)

            qt_sb = qkv.tile([P, 4, S], f16)      # QT feature-major
            kt_sb = qkv.tile([P, 2, S], f16)      # KT feature-major
            vE_sb = qkv.tile([P, 16, D + 1], bf16)  # V token-major + ones col

            # DMA: large per-partition-contiguous descriptors, issued in
            # rounds of k-chunks so the ts=0 k-outer projection pass can
            # consume chunk k as soon as its round lands.
            ht0 = hpool.tile([P, KC, TCW], f16, tag="ht", name="ht0")
            for ri, (a, bnd) in enumerate([(0, 2), (2, 7), (7, 12), (12, KC)]):
                nc.sync.dma_start(ht0[:, a:bnd, :], h4[:, 0, a:bnd, :])
                nc.sync.dma_start(wk_sb[:, a:bnd, :], wk4[:, a:bnd, :])
                nc.sync.dma_start(wv_sb[:, a:bnd, :], wv4[:, a:bnd, :])
                nc.sync.dma_start(wq_sb[:, a:bnd, :], wq4[:, a:bnd, :])
                if ri == 1:
                    nc.sync.dma_start(cos_sb[:], cosT[:, :])
                    nc.sync.dma_start(sin_sb[:], sinT[:, :])
            nc.sync.dma_start(wo_sb[:], wo4[:, :, :])

            # small one-time SBUF setup (gpsimd) + PE warm-up during DMA wait
            make_identity(nc, ident_bf[:])
            nc.gpsimd.memset(wtile[:], 0.0)
            nc.gpsimd.memset(negb[:], -SOFTCAP)
            nc.gpsimd.memset(vE_sb[:, :, D], 1.0)
            # transposed-orientation additive masks (0 keep / -3 mask):
            # value(p=dk, head j, f=dq-in-pair) independent of head (coef 0).
            for m_t, base, csign in (
                (mA, -1, 1),     # oldest block: edge for half A, all-mask B
                (mB, 127, 1),    # second block: keep A, edge for half B
                (mC, 0, -1),     # diag block of half A; keep for half B
                (mD, -128, -1),  # diag block of half B; all-mask for half A
            ):
                nc.gpsimd.memset(m_t[:], 0.0)
                nc.gpsimd.affine_select(
                    out=m_t[:], in_=m_t[:],
                    compare_op=mybir.AluOpType.is_ge, fill=-3.0,
                    base=base, pattern=[[0, 2], [-csign, 256]],
                    channel_multiplier=csign)
            warm_ps = psA.tile([P, 512], f32, tag="bank", name="warm")
            for i in range(20):
                nc.tensor.matmul(warm_ps[:, 0:P], wtile[:], wtile[:],
                                 start=True, stop=True)

            def rope_pair(ps_lo, ps_hi, dst, m_lo, m_hi, ts):
                tsl = slice(ts * TCW, (ts + 1) * TCW)
                cs, sn = cos_sb[:, tsl], sin_sb[:, tsl]
                t1 = work.tile([P, TCW], f16, tag="rope_t1")
                t2 = work.tile([P, TCW], f16, tag="rope_t2")
                nc.vector.tensor_mul(t1[:], ps_hi[:], sn)
                nc.vector.tensor_mul(t2[:], ps_lo[:], sn)
                lo = dst[:, m_lo, tsl]
                hi = dst[:, m_hi, tsl]
                nc.vector.tensor_mul(lo, ps_lo[:], cs)
                nc.vector.tensor_sub(lo, lo, t1[:])
                nc.vector.tensor_mul(hi, ps_hi[:], cs)
                nc.vector.tensor_add(hi, hi, t2[:])

            def proj_chunk0(ht):
                """ts=0 projection, k-outer: all 8 PSUM groups accumulate in
                lockstep with DMA chunk arrival (Q pair 1 in a second pass)."""
                pq0 = [psA.tile([P, 512], f32, tag="bank", name=f"pq0_{i}")
                       for i in range(2)]
                pk = [psA.tile([P, 512], f32, tag="bank", name=f"pk0_{i}")
                      for i in range(2)]
                pv4 = [psA.tile([P, 512], f32, tag="bank", name=f"pv0_{mt}")
                       for mt in range(4)]
                for k in range(KC):
                    st, sp = (k == 0), (k == KC - 1)
                    for i in range(2):
                        nc.tensor.matmul(
                            pq0[i][:], wq_sb[:, k, i * P:(i + 1) * P],
                            ht[:, k, :], start=st, stop=sp)
                    for i in range(2):
                        nc.tensor.matmul(
                            pk[i][:], wk_sb[:, k, i * P:(i + 1) * P],
                            ht[:, k, :], start=st, stop=sp)
                    for mt in range(4):
                        nc.tensor.matmul(
                            pv4[mt][:, :D], ht[:, k, mt * P:(mt + 1) * P],
                            wv_sb[:, k, :], start=st, stop=sp)
                rope_pair(pq0[0], pq0[1], qt_sb, 0, 1, 0)
                rope_pair(pk[0], pk[1], kt_sb, 0, 1, 0)
                for mt in range(4):
                    nc.scalar.copy(vE_sb[:, mt, 0:D], pv4[mt][:, :D])
                pq1 = [psA.tile([P, 512], f32, tag="bank", name=f"pq1_{i}")
                       for i in range(2)]
                for i in range(2):
                    for k in range(KC):
                        nc.tensor.matmul(
                            pq1[i][:], wq_sb[:, k, (2 + i) * P:(3 + i) * P],
                            ht[:, k, :], start=(k == 0), stop=(k == KC - 1))
                rope_pair(pq1[0], pq1[1], qt_sb, 2, 3, 0)

            def proj_chunk(ts, ht):
                for pair in range(2):
                    pq = [psA.tile([P, 512], f32, tag="bank",
                                   name=f"pq{ts}_{pair}_{i}") for i in range(2)]
                    for i in range(2):
                        m = 2 * pair + i
                        for k in range(KC):
                            nc.tensor.matmul(
                                pq[i][:], wq_sb[:, k, m * P:(m + 1) * P],
                                ht[:, k, :], start=(k == 0), stop=(k == KC - 1))
                    rope_pair(pq[0], pq[1], qt_sb, 2 * pair, 2 * pair + 1, ts)
                pk = [psA.tile([P, 512], f32, tag="bank", name=f"pk{ts}_{i}")
                      for i in range(2)]
                for i in range(2):
                    for k in range(KC):
                        nc.tensor.matmul(
                            pk[i][:], wk_sb[:, k, i * P:(i + 1) * P],
                            ht[:, k, :], start=(k == 0), stop=(k == KC - 1))
                rope_pair(pk[0], pk[1], kt_sb, 0, 1, ts)
                for mt in range(4):
                    pv = psA.tile([P, 512], f32, tag="bank")
                    for k in range(KC):
                        nc.tensor.matmul(
                            pv[:, :D], ht[:, k, mt * P:(mt + 1) * P],
                            wv_sb[:, k, :], start=(k == 0), stop=(k == KC - 1))
                    nc.scalar.copy(vE_sb[:, ts * 4 + mt, 0:D], pv[:, :D])

            def emit_wo(prev, last=False):
                """Wo partials for finished query blocks [(atT, q0), ...]."""
                if prev is None:
                    return
                for atT, q0 in prev:
                    osb = work.tile([P, HID], f16, tag="osb", name=f"osb{q0}")
                    hg0 = 0
                    for gi, hgw in enumerate(HG_WIDTHS):
                        po = psA.tile([P, 512], f32, tag="bank",
                                      name=f"po{q0}_{gi}")
                        for m in range(4):
                            nc.tensor.matmul(
                                po[:, :hgw], atT[:, m, :],
                                wo_sb[:, m, hg0:hg0 + hgw],
                                start=(m == 0), stop=(m == 3))
                        if gi % 2 == 0:
                            nc.vector.tensor_copy(osb[:, hg0:hg0 + hgw],
                                                  po[:, :hgw])
                        else:
                            nc.scalar.copy(osb[:, hg0:hg0 + hgw], po[:, :hgw])
                        if last:
                            nc.sync.dma_start(out[q0:q0 + P, hg0:hg0 + hgw],
                                              osb[:, hg0:hg0 + hgw])
                        hg0 += hgw
                    if not last:
                        nc.sync.dma_start(out[q0:q0 + P, :], osb[:])

            def emit_scores_pair(p):
                """Transposed softcapped scores for query blocks 2p, 2p+1."""
                j0 = max(0, 2 * p - 8)
                nkb = 2 * p + 2 - j0
                qsl = slice(2 * p * P, (2 * p + 2) * P)
                tbufT = scpool.tile([P, 10, 2, 256], f32, tag="tbufT")
                ptb = ptpool.tile([P, 10, 2, 256], bf16, tag="ptb",
                                  name=f"ptb{p}")
                for jl in range(nkb):
                    kb = j0 + jl
                    ps = psA.tile([P, 2, 256], f32, tag="bank",
                                  name=f"ps{p}_{jl}")
                    psv = ps
                    for h in range(2):
                        for i in range(2):
                            nc.tensor.matmul(
                                psv[:, h, :],
                                kt_sb[:, i, kb * P:(kb + 1) * P],
                                qt_sb[:, 2 * h + i, qsl],
                                start=(i == 0), stop=(i == 1))
                    nc.scalar.activation(
                        tbufT[:, jl, :, :], psv[:, :, :], AF.Tanh,
                        scale=SCALING / SOFTCAP)
                if j0 == 2 * p - 8:
                    nc.vector.tensor_add(tbufT[:, 0], tbufT[:, 0], mA[:])
                    nc.vector.tensor_add(tbufT[:, 1], tbufT[:, 1], mB[:])
                nc.vector.tensor_add(tbufT[:, nkb - 2], tbufT[:, nkb - 2],
                                     mC[:])
                nc.vector.tensor_add(tbufT[:, nkb - 1], tbufT[:, nkb - 1],
                                     mD[:])
                nc.scalar.activation(
                    ptb[:, 0:nkb], tbufT[:, 0:nkb], AF.Exp,
                    scale=SOFTCAP, bias=negb[:])
                return (p, j0, nkb, ptb)

            def finish_pair(pend):
                """AV + normalization + attnT for both halves of a pair."""
                p, j0, nkb, ptb = pend
                at = work.tile([P, 2, 2 * D], bf16, tag="at", name=f"at{p}")
                res = []
                for s in range(2):
                    qi = 2 * p + s
                    kb_lo = max(0, qi - 8)
                    nb = qi - kb_lo + 1
                    for h in range(2):
                        pav = psA.tile([P, 512], f32, tag="bank",
                                       name=f"pav{p}_{s}_{h}")
                        for n in range(nb):
                            kb = kb_lo + n
                            nc.tensor.matmul(
                                pav[:, 0:D + 1],
                                ptb[:, kb - j0, h, s * P:(s + 1) * P],
                                vE_sb[:, kb, :],
                                start=(n == 0), stop=(n == nb - 1))
                        recip = small.tile([P, 1], f32, tag="recip")
                        nc.vector.reciprocal(recip[:], pav[:, D:D + 1])
                        nc.vector.tensor_scalar_mul(
                            at[:, s, h * D:(h + 1) * D], pav[:, 0:D],
                            recip[:])
                    attps = psA.tile([P, 512], bf16, tag="bank",
                                     name=f"att{p}_{s}")
                    for m in range(4):
                        nc.tensor.transpose(
                            attps[:, m * P:(m + 1) * P],
                            at[:, s, m * P:(m + 1) * P], ident_bf[:])
                    atT = att3.tile([P, 4, P], bf16, tag="atT",
                                    name=f"atT{p}_{s}")
                    nc.scalar.copy(atT[:], attps[:])
                    res.append((atT, qi * P))
                return res

            # ---------------- merged pipeline (2-deep) ----------------
            prev = None      # finished halves awaiting Wo
            pend = None      # scored pair awaiting softmax/AV
            ht_cur = ht0
            ht_next = None
            for p in range(NPAIR):
                if p % 2 == 0:
                    ts = p // 2
                    if ts + 1 < NTC:
                        ht_next = hpool.tile([P, KC, TCW], f16, tag="ht",
                                             name=f"ht{ts + 1}")
                        nc.sync.dma_start(ht_next[:],
                                          h4[:, ts + 1, :, :])
                    if ts == 0:
                        proj_chunk0(ht_cur)
                    else:
                        proj_chunk(ts, ht_cur)
                    ht_cur = ht_next
                sc = emit_scores_pair(p)
                emit_wo(prev)
                prev = None
                if pend is not None:
                    prev = finish_pair(pend)
                pend = sc
            emit_wo(prev)
            prev = finish_pair(pend)
            emit_wo(prev, last=True)

    nc.compile()
    return nc


def _get_nc():
    if "nc" not in _CACHED:
        _CACHED["nc"] = _build_nc()
    return _CACHED["nc"]


def kernel(hidden_states, Wq, Wk, Wv, Wo, position_ids):
    import ml_dtypes
    from concourse.bass_utils import run_bass_kernel_spmd

    hidden_states = np.asarray(hidden_states)
    Wq, Wk, Wv, Wo = (np.asarray(a) for a in (Wq, Wk, Wv, Wo))
    position_ids = np.asarray(position_ids)

    inv_freq = 1.0 / (THETA ** (np.arange(0, D, 2, dtype=np.float64) / D))
    freqs = position_ids.astype(np.float64)[None, :] * inv_freq[:, None]
    cos_t = np.cos(freqs).astype(np.float16)
    sin_t = np.sin(freqs).astype(np.float16)

    in_maps = []
    for c in range(8):
        b, r = divmod(c, 4)
        hT = hidden_states[b].T.astype(np.float16)          # [HID, S]
        h4 = np.ascontiguousarray(
            hT.reshape(KC, P, NTC, TCW).transpose(1, 2, 0, 3))
        wq4 = np.ascontiguousarray(
            Wq[512 * r:512 * (r + 1)].T.astype(np.float16)
            .reshape(KC, P, 2 * D).transpose(1, 0, 2))
        wk4 = np.ascontiguousarray(
            Wk[256 * r:256 * (r + 1)].T.astype(np.float16)
            .reshape(KC, P, D).transpose(1, 0, 2))
        wv4 = np.ascontiguousarray(
            Wv[256 * r:256 * (r + 1)].T.astype(np.float16)
            .reshape(KC, P, D).transpose(1, 0, 2))
        wo4 = np.ascontiguousarray(
            Wo[:, 512 * r:512 * (r + 1)].T.astype(ml_dtypes.bfloat16)
            .reshape(4, P, HID).transpose(1, 0, 2))
        in_maps.append({
            "h4": h4, "wq4": wq4, "wk4": wk4, "wv4": wv4, "wo4": wo4,
            "cosT": cos_t, "sinT": sin_t,
        })

    _CACHED["last_in_maps"] = in_maps
    globals()["_last_in_maps"] = in_maps
    res = run_bass_kernel_spmd(_get_nc(), in_maps, core_ids=list(range(8)))
    parts = [r["out"].astype(np.float32) for r in res.results]
    full = np.stack([
        parts[0] + parts[1] + parts[2] + parts[3],
        parts[4] + parts[5] + parts[6] + parts[7],
    ])
    return full
